# revision 14
# baseline (speedup 1.0000x reference)
"""Trainium2 Bass kernel for nn_CompositionalPID (moe_routing).

Sharding: data-parallel over batch B=16 across 8 NeuronCores (2 batches/core).
All heavy compute on-device; host only shards inputs, gathers the three
composer depth candidates + per-batch gate sigmoids, and selects the
candidate according to the global gate means (avoids cross-core collectives).

Layout scheme per core:
  - activations row-major [rows<=128 partitions, features free] for LayerNorm
    (bn_stats idiom) and row-wise ops
  - every GEMM consumes the activation transposed (feature-major) as the
    PE stationary operand (lhsT); transposes via PE is_transpose + copyback,
    which also performs the float32r rounding required by the fp32r fast path
  - big GEMMs run in float32r (1 cycle/row vs 4 for fp32)
"""
import sys

sys.path.insert(0, "/opt/trn_rl_repo")

from contextlib import ExitStack

import numpy as np

import concourse.bacc as bacc
import concourse.bass as bass
import concourse.tile as tile
from concourse import mybir
from concourse.bass_utils import run_bass_kernel_spmd
from concourse.masks import make_identity

F32 = mybir.dt.float32
F32R = mybir.dt.float32r
AX = mybir.AxisListType
ALU = mybir.AluOpType
ACTF = mybir.ActivationFunctionType

P = 128
D = 768
FF = 2048
H = 8
HD = 96
NC8 = 8
B, N, L = 16, 256, 64
NCORES = 8
BL = B // NCORES          # batches per core = 2
ROWS = BL * NC8 * N       # operator rows per core = 4096
CROWS = BL * N            # composer rows per core = 512
KD = D // P               # 6
K2D = 2 * D // P          # 12

TRACE = False             # set by test.py for profiling runs
_CACHE = {}


def _bcast_ap(vec_ap, nparts):
    """[F] dram vector -> [nparts, F] partition-broadcast DMA source AP."""
    return bass.AP(tensor=vec_ap.tensor, offset=vec_ap.offset,
                   ap=[[0, nparts], *vec_ap.ap])


class Ctx:
    pass


def _ln_stats(nc, cx, pool, src_slices, nrows=P):
    """bn_stats over (ap slices, each <=512 wide) -> (mean, rstd) [P,1] APs."""
    nsub = len(src_slices)
    stats = pool.tile([P, nsub, 6], F32, tag="ln_stats")
    for i, sl in enumerate(src_slices):
        nc.vector.bn_stats(out=stats[:nrows, i, :], in_=sl)
    mv = pool.tile([P, 2], F32, tag="ln_mv")
    nc.vector.bn_aggr(out=mv[:nrows], in_=stats[:nrows])
    std = pool.tile([P, 1], F32, tag="ln_std")
    nc.scalar.activation(out=std[:nrows], in_=mv[:nrows, 1:2], func=ACTF.Sqrt,
                         bias=cx.eps[:nrows], scale=1.0)
    rstd = pool.tile([P, 1], F32, tag="ln_rstd")
    nc.vector.reciprocal(out=rstd[:nrows], in_=std[:nrows])
    return mv[:, 0:1], rstd


def _tp128(nc, cx, psum_pool, out_fm, in_rm, nblocks, relu=False,
           src_col0=0, dst_col0=0):
    """Transpose in_rm[:, src_col0+b*128 : ...] 128x128 blocks into
    out_fm[:, b, dst_col0:dst_col0+128]; cast-copy rounds to fp32r."""
    for b0 in range(0, nblocks, 4):
        nb = min(4, nblocks - b0)
        pst = psum_pool.tile([P, 4 * P], F32, tag="trp")
        for bb in range(nb):
            nc.tensor.transpose(
                pst[:, bb * P:(bb + 1) * P],
                in_rm[:, src_col0 + (b0 + bb) * P: src_col0 + (b0 + bb + 1) * P],
                cx.ident)
        for bb in range(nb):
            dst = out_fm[:, b0 + bb, dst_col0:dst_col0 + P]
            if relu:
                nc.scalar.activation(out=dst, in_=pst[:, bb * P:(bb + 1) * P],
                                     func=ACTF.Relu)
            else:
                nc.any.tensor_copy(out=dst, in_=pst[:, bb * P:(bb + 1) * P])


def _gemm(nc, psum_out, lhsT_fn, w_fm, ksubs, out_w, mrows=P):
    """psum_out[:mrows, :out_w] = sum_ks lhsT(ks).T @ w_fm[:, ks, :]."""
    for n0 in range(0, out_w, 512):
        nn = min(512, out_w - n0)
        for ks in range(ksubs):
            nc.tensor.matmul(psum_out[:mrows, n0:n0 + nn],
                             lhsT=lhsT_fn(ks),
                             rhs=w_fm[:, ks, n0:n0 + nn],
                             start=(ks == 0), stop=(ks == ksubs - 1))


def _softmax_rows(nc, pool, sl, nrows, tagp):
    """In-place softmax over the free dim of sl [nrows, F<=512]."""
    rmax = pool.tile([P, 1], F32, tag=f"{tagp}_rmax")
    nc.vector.tensor_reduce(out=rmax[:nrows], in_=sl, axis=AX.X, op=ALU.max)
    nmax = pool.tile([P, 1], F32, tag=f"{tagp}_nmax")
    nc.vector.tensor_scalar_mul(nmax[:nrows], rmax[:nrows], -1.0)
    nc.scalar.activation(out=sl, in_=sl, func=ACTF.Exp, bias=nmax[:nrows],
                         scale=1.0)
    rsum = pool.tile([P, 1], F32, tag=f"{tagp}_rsum")
    nc.vector.tensor_reduce(out=rsum[:nrows], in_=sl, axis=AX.X, op=ALU.add)
    rrec = pool.tile([P, 1], F32, tag=f"{tagp}_rrec")
    nc.vector.reciprocal(rrec[:nrows], rsum[:nrows])
    nc.vector.tensor_scalar_mul(sl, sl, rrec[:nrows])


# ------------------------------------------------------------- build phases
def _build_encoder(nc, tc, cx):
    """3-layer post-norm transformer; leaves final x_rm in cx.x_rm (glob)."""
    with ExitStack() as stk:
        local = stk.enter_context(tc.tile_pool(name="enc_local", bufs=1))
        wpool = stk.enter_context(tc.tile_pool(name="enc_w", bufs=1))
        big = stk.enter_context(tc.tile_pool(name="enc_big", bufs=1))
        work = stk.enter_context(tc.tile_pool(name="enc_work", bufs=2))
        ps = stk.enter_context(tc.tile_pool(name="enc_ps", bufs=2, space="PSUM"))
        att = stk.enter_context(tc.tile_pool(name="enc_att", bufs=1, space="PSUM"))
        att2 = stk.enter_context(tc.tile_pool(name="enc_att2", bufs=2, space="PSUM"))

        x_rm = local.tile([P, D], F32, tag="enc_x_in")
        nc.sync.dma_start(out=x_rm, in_=cx.t_text[:, :])

        mb = []
        for b2 in range(BL):
            t = local.tile([64, 64], F32, tag=f"maskb{b2}")
            nc.sync.dma_start(out=t, in_=bass.AP(
                tensor=cx.t_maskb.tensor, offset=cx.t_maskb.offset + b2 * 64,
                ap=[[0, 64], [1, 64]]))
            mb.append(t)

        general = not cx.enc_trivial
        for li in range(3):
            gw = {}
            if general:
                for nm, width in (("ln1w", D), ("ln1b", D), ("ln2w", D),
                                  ("ln2b", D), ("qkvb", 3 * D), ("outb", D),
                                  ("ff1b", FF), ("ff2b", D)):
                    t = work.tile([P, width], F32, tag=f"egw_{nm}")
                    nc.gpsimd.dma_start(out=t, in_=_bcast_ap(cx.t_enc[li][nm], P))
                    gw[nm] = t

            # ---- qkv
            xT = work.tile([P, KD, P], F32R, tag="enc_xT")
            _tp128(nc, cx, ps, xT, x_rm, KD)
            qkv_w = wpool.tile([P, KD, 3 * D], F32R, tag="encw")
            nc.sync.dma_start(out=qkv_w, in_=cx.t_enc[li]["qkvwT"].rearrange(
                "(ks p) o -> p ks o", p=P))
            qkv_sb = big.tile([P, 3 * D], F32, tag="enc_qkv")
            for n0 in range(0, 3 * D, 512):
                nn = min(512, 3 * D - n0)
                pq = ps.tile([P, 512], F32, tag="enc_psq")
                for ks in range(KD):
                    nc.tensor.matmul(pq[:, :nn], lhsT=xT[:, ks, :],
                                     rhs=qkv_w[:, ks, n0:n0 + nn],
                                     start=(ks == 0), stop=(ks == KD - 1))
                if general:
                    nc.vector.tensor_add(out=qkv_sb[:, n0:n0 + nn],
                                         in0=pq[:, :nn],
                                         in1=gw["qkvb"][:, n0:n0 + nn])
                else:
                    nc.any.tensor_copy(out=qkv_sb[:, n0:n0 + nn], in_=pq[:, :nn])

            # ---- attention per (h, b); all per-head tiles live at base
            # partition 0 (transpose outputs must land at PSUM partition 0),
            # so batch 1's V is staged to partition 0 via a small DMA and the
            # two batches are merged later at the feature-major oT stage.
            v1 = big.tile([64, D], F32, tag="enc_v1")
            nc.sync.dma_start(out=v1, in_=qkv_sb[64:128, 2 * D:3 * D])
            o_b = [big.tile([64, D], F32, tag="enc_ob0", name="ob0"),
                   big.tile([64, D], F32, tag="enc_ob1", name="ob1")]
            for h in range(H):
                for b2 in range(BL):
                    prow = b2 * 64
                    qT = work.tile([HD, 64], F32, tag="enc_qT")
                    kT = work.tile([HD, 64], F32, tag="enc_kT")
                    for (dst, col0) in ((qT, h * HD), (kT, D + h * HD)):
                        pt = att.tile([HD, 64], F32, tag="enc_ptq")
                        nc.tensor.transpose(
                            pt, qkv_sb[prow:prow + 64, col0:col0 + HD],
                            cx.ident[prow:prow + 64, prow:prow + 64])
                        nc.any.tensor_copy(out=dst, in_=pt)
                    s_ps = att2.tile([64, 64], F32, tag="enc_sa")
                    nc.tensor.matmul(s_ps, lhsT=qT, rhs=kT,
                                     start=True, stop=True)
                    s_sb = work.tile([64, 64], F32, tag="enc_ssb")
                    nc.vector.scalar_tensor_tensor(
                        out=s_sb, in0=s_ps,
                        scalar=float(1.0 / np.sqrt(HD)),
                        in1=mb[b2], op0=ALU.mult, op1=ALU.add)
                    _softmax_rows(nc, work, s_sb, 64, "enc_sm")
                    aT_ps = att2.tile([64, 64], F32, tag="enc_sa")
                    nc.tensor.transpose(aT_ps, s_sb, cx.ident[:64, :64])
                    aT = work.tile([64, 64], F32, tag="enc_aT")
                    nc.any.tensor_copy(out=aT, in_=aT_ps)
                    vsrc = (qkv_sb[0:64, 2 * D + h * HD:2 * D + (h + 1) * HD]
                            if b2 == 0 else v1[:, h * HD:(h + 1) * HD])
                    o_ps = att.tile([64, HD], F32, tag="enc_ops")
                    nc.tensor.matmul(o_ps, lhsT=aT, rhs=vsrc,
                                     start=True, stop=True)
                    nc.any.tensor_copy(out=o_b[b2][:, h * HD:(h + 1) * HD],
                                       in_=o_ps)

            # ---- attn out-proj + residual + LN1 (merge batches into oT cols)
            oT = work.tile([P, KD, P], F32R, tag="enc_oT")
            for b2 in range(BL):
                for kb in range(KD):
                    pt4 = ps.tile([P, 4 * P], F32, tag="trp")
                    nc.tensor.transpose(pt4[:, 0:64],
                                        o_b[b2][:, kb * P:(kb + 1) * P],
                                        cx.ident[:64, :64])
                    nc.any.tensor_copy(out=oT[:, kb, b2 * 64:(b2 + 1) * 64],
                                       in_=pt4[:, 0:64])
            out_w = wpool.tile([P, KD, D], F32R, tag="encw")
            nc.sync.dma_start(out=out_w, in_=cx.t_enc[li]["outwT"].rearrange(
                "(ks p) o -> p ks o", p=P))
            t1 = big.tile([P, D], F32, tag="enc_t1")
            for n0 in (0, 512):
                nn = min(512, D - n0)
                pq = ps.tile([P, 512], F32, tag="enc_psq")
                for ks in range(KD):
                    nc.tensor.matmul(pq[:, :nn], lhsT=oT[:, ks, :],
                                     rhs=out_w[:, ks, n0:n0 + nn],
                                     start=(ks == 0), stop=(ks == KD - 1))
                if general:
                    nc.vector.tensor_add(out=t1[:, n0:n0 + nn], in0=pq[:, :nn],
                                         in1=gw["outb"][:, n0:n0 + nn])
                    nc.vector.tensor_add(out=t1[:, n0:n0 + nn],
                                         in0=t1[:, n0:n0 + nn],
                                         in1=x_rm[:, n0:n0 + nn])
                else:
                    nc.vector.tensor_add(out=t1[:, n0:n0 + nn], in0=pq[:, :nn],
                                         in1=x_rm[:, n0:n0 + nn])
            mean, rstd = _ln_stats(nc, cx, work, [t1[:, 0:512], t1[:, 512:768]])
            x1 = big.tile([P, D], F32, tag="enc_x1")
            nc.vector.tensor_scalar(out=x1, in0=t1, scalar1=mean, scalar2=rstd,
                                    op0=ALU.subtract, op1=ALU.mult)
            if general:
                nc.vector.tensor_mul(out=x1, in0=x1, in1=gw["ln1w"])
                nc.vector.tensor_add(out=x1, in0=x1, in1=gw["ln1b"])

            # ---- FF
            x1T = work.tile([P, KD, P], F32R, tag="enc_xT")
            _tp128(nc, cx, ps, x1T, x1, KD)
            ff1_w = wpool.tile([P, KD, FF], F32R, tag="encw")
            nc.sync.dma_start(out=ff1_w, in_=cx.t_enc[li]["ff1wT"].rearrange(
                "(ks p) o -> p ks o", p=P))
            h_sb = big.tile([P, FF], F32, tag="enc_hsb")
            for n0 in range(0, FF, 512):
                ph = ps.tile([P, 512], F32, tag="enc_psq")
                for ks in range(KD):
                    nc.tensor.matmul(ph, lhsT=x1T[:, ks, :],
                                     rhs=ff1_w[:, ks, n0:n0 + 512],
                                     start=(ks == 0), stop=(ks == KD - 1))
                if general:
                    nc.vector.tensor_add(out=h_sb[:, n0:n0 + 512], in0=ph,
                                         in1=gw["ff1b"][:, n0:n0 + 512])
                    nc.vector.tensor_scalar_max(h_sb[:, n0:n0 + 512],
                                                h_sb[:, n0:n0 + 512], 0.0)
                else:
                    nc.scalar.activation(out=h_sb[:, n0:n0 + 512], in_=ph,
                                         func=ACTF.Relu)
            hT = work.tile([P, FF // P, P], F32R, tag="enc_hT")
            _tp128(nc, cx, ps, hT, h_sb, FF // P)
            ff2_w = wpool.tile([P, FF // P, D], F32R, tag="encw")
            nc.sync.dma_start(out=ff2_w, in_=cx.t_enc[li]["ff2wT"].rearrange(
                "(ks p) o -> p ks o", p=P))
            t2 = big.tile([P, D], F32, tag="enc_t1")
            for n0 in (0, 512):
                nn = min(512, D - n0)
                pq = ps.tile([P, 512], F32, tag="enc_psq")
                for ks in range(FF // P):
                    nc.tensor.matmul(pq[:, :nn], lhsT=hT[:, ks, :],
                                     rhs=ff2_w[:, ks, n0:n0 + nn],
                                     start=(ks == 0), stop=(ks == FF // P - 1))
                if general:
                    nc.vector.tensor_add(out=t2[:, n0:n0 + nn], in0=pq[:, :nn],
                                         in1=gw["ff2b"][:, n0:n0 + nn])
                    nc.vector.tensor_add(out=t2[:, n0:n0 + nn],
                                         in0=t2[:, n0:n0 + nn],
                                         in1=x1[:, n0:n0 + nn])
                else:
                    nc.vector.tensor_add(out=t2[:, n0:n0 + nn], in0=pq[:, :nn],
                                         in1=x1[:, n0:n0 + nn])
            mean2, rstd2 = _ln_stats(nc, cx, work,
                                     [t2[:, 0:512], t2[:, 512:768]])
            if li == 2:
                x_out = cx.glob.tile([P, D], F32, tag="x_rm_final",
                                     name="x_rm_final")
            else:
                x_out = local.tile([P, D], F32, tag=f"enc_x_{li}",
                                   name=f"enc_x_{li}")
            nc.vector.tensor_scalar(out=x_out, in0=t2, scalar1=mean2,
                                    scalar2=rstd2, op0=ALU.subtract,
                                    op1=ALU.mult)
            if general:
                nc.vector.tensor_mul(out=x_out, in0=x_out, in1=gw["ln2w"])
                nc.vector.tensor_add(out=x_out, in0=x_out, in1=gw["ln2b"])
            x_rm = x_out
    cx.x_rm = x_rm


def _build_head(nc, tc, cx):
    """pooled, cw softmax, cwrep/cwT, sel_in pooled columns (into cx.*)."""
    with ExitStack() as stk:
        pool = stk.enter_context(tc.tile_pool(name="head", bufs=1))
        ps = stk.enter_context(tc.tile_pool(name="head_ps", bufs=1, space="PSUM"))
        x_rm = cx.x_rm

        pp = ps.tile([P, D], F32, tag="h_pp")
        for b2 in range(BL):
            for n0 in (0, 512):
                nn = min(512, D - n0)
                nc.tensor.matmul(pp[b2 * 64:b2 * 64 + 1, n0:n0 + nn],
                                 lhsT=cx.ones_col[b2 * 64:(b2 + 1) * 64, :],
                                 rhs=x_rm[b2 * 64:(b2 + 1) * 64, n0:n0 + nn],
                                 start=True, stop=True)
        pbig = pool.tile([P, D], F32)
        nc.scalar.activation(out=pbig, in_=pp, func=ACTF.Copy, scale=1.0 / L)
        pooled = cx.glob.tile([BL, D], F32, tag="pooled")
        for b2 in range(BL):
            nc.sync.dma_start(out=pooled[b2:b2 + 1, :],
                              in_=pbig[b2 * 64:b2 * 64 + 1, :])

        pooledT = pool.tile([P, KD, BL], F32R)
        for kb in range(KD):
            pt = ps.tile([P, BL], F32, tag="h_pt")
            nc.tensor.transpose(pt, pooled[:, kb * P:(kb + 1) * P],
                                cx.ident[:BL, :BL])
            nc.any.tensor_copy(out=pooledT[:, kb, :], in_=pt)
        w1 = pool.tile([P, KD, 2 * D], F32R)
        nc.sync.dma_start(out=w1,
                          in_=cx.t_og_w1T.rearrange("(ks p) o -> p ks o", p=P))
        g1 = ps.tile([BL, 2 * D], F32, tag="h_g1")
        _gemm(nc, g1, lambda ks: pooledT[:, ks, :], w1, KD, 2 * D, mrows=BL)
        r1 = pool.tile([BL, 2 * D], F32)
        b1bc = pool.tile([BL, 2 * D], F32)
        nc.gpsimd.dma_start(out=b1bc, in_=_bcast_ap(cx.t_og_b1, BL))
        nc.vector.tensor_add(out=r1, in0=g1, in1=b1bc)
        nc.vector.tensor_scalar_max(r1, r1, 0.0)
        r1T = pool.tile([P, K2D, BL], F32R)
        for kb in range(K2D):
            pt = ps.tile([P, BL], F32, tag="h_pt")
            nc.tensor.transpose(pt, r1[:, kb * P:(kb + 1) * P],
                                cx.ident[:BL, :BL])
            nc.any.tensor_copy(out=r1T[:, kb, :], in_=pt)
        w2 = pool.tile([P, K2D, 64], F32R)
        nc.sync.dma_start(out=w2,
                          in_=cx.t_og_w2T.rearrange("(ks p) o -> p ks o", p=P))
        g2 = ps.tile([BL, 64], F32, tag="h_g2")
        _gemm(nc, g2, lambda ks: r1T[:, ks, :], w2, K2D, 64, mrows=BL)
        cm = pool.tile([BL, 64], F32)
        b2bc = pool.tile([BL, 64], F32)
        nc.gpsimd.dma_start(out=b2bc, in_=_bcast_ap(cx.t_og_b2, BL))
        nc.vector.tensor_add(out=cm, in0=g2, in1=b2bc)
        for g in range(NC8):
            _softmax_rows(nc, pool, cm[:, g * 8:(g + 1) * 8], BL, "h_cw")

        nc.sync.dma_start(out=cx.t_cmsc[:, :], in_=cm)
        cwflat = pool.tile([1, P], F32)
        nc.sync.dma_start(out=cwflat,
                          in_=cx.t_cmsc.rearrange("b f -> (b f)")[None, :])
        cwrep = cx.glob.tile([P, P], F32, tag="cwrep")
        nc.gpsimd.partition_broadcast(cwrep, cwflat)

        cwT = []
        for b2 in range(BL):
            t = cx.glob.tile([NC8, NC8], F32, tag=f"cwT{b2}")
            nc.sync.dma_start(out=t,
                              in_=cx.t_cmsc[b2, :].rearrange("(i j) -> j i", j=8))
            cwT.append(t)

        selin = cx.glob.tile([2 * NC8, 2 * D], F32, tag="selin")
        for b2 in range(BL):
            for i in range(NC8):
                nc.sync.dma_start(
                    out=selin[b2 * NC8 + i:b2 * NC8 + i + 1, D:2 * D],
                    in_=pooled[b2:b2 + 1, :])
    cx.cwrep, cx.cwT, cx.selin = cwrep, cwT, selin


def _build_combined(nc, tc, cx):
    """comp (b,j,n layout) -> combT dram f32r; compmean -> selin[:, :768]."""
    with ExitStack() as stk:
        pool = stk.enter_context(tc.tile_pool(name="c3", bufs=3))
        bpool = stk.enter_context(tc.tile_pool(name="c3b", bufs=1))
        ps = stk.enter_context(tc.tile_pool(name="c3_ps", bufs=2, space="PSUM"))
        pmps = stk.enter_context(tc.tile_pool(name="c3_pm", bufs=1, space="PSUM"))

        for b2 in range(BL):
            compT_b = bpool.tile([P, KD, NC8 * N], F32, tag="compT")
            pmean = bpool.tile([NC8, D], F32, tag=f"pmean{b2}")
            for j in range(NC8):
                pm = pmps.tile([1, D], F32, tag="c3_pmps")
                for hh in range(2):
                    cidx = (b2 * NC8 + j) * 2 + hh
                    chunk = pool.tile([P, D], F32, tag="c3_chunk")
                    nc.sync.dma_start(out=chunk,
                                      in_=cx.t_comp[cidx * P:(cidx + 1) * P, :])
                    _tp128(nc, cx, ps, compT_b, chunk, KD,
                           dst_col0=j * N + hh * P)
                    for n0 in (0, 512):
                        nn = min(512, D - n0)
                        nc.tensor.matmul(pm[:, n0:n0 + nn], lhsT=cx.ones_col,
                                         rhs=chunk[:, n0:n0 + nn],
                                         start=(hh == 0), stop=(hh == 1))
                pmsb = pool.tile([1, D], F32, tag="c3_pmsb")
                nc.any.tensor_copy(out=pmsb, in_=pm)
                nc.sync.dma_start(out=pmean[j:j + 1, :], in_=pmsb)
            for i in range(NC8):
                mix = pool.tile([P, KD, N], F32R, tag="c3_mix")
                for j in range(NC8):
                    src = compT_b[:, :, j * N:(j + 1) * N]
                    ci = b2 * 64 + i * 8 + j
                    sc = cx.cwrep[:, ci:ci + 1]
                    if j == 0:
                        nc.vector.tensor_scalar_mul(mix, src, sc)
                    else:
                        nc.vector.scalar_tensor_tensor(
                            out=mix, in0=src, scalar=sc, in1=mix,
                            op0=ALU.mult, op1=ALU.add)
                col0 = (b2 * NC8 + i) * N
                nc.sync.dma_start(
                    out=cx.t_combT[:, :, col0:col0 + N].rearrange(
                        "ks p r -> p ks r"),
                    in_=mix)
            sm = pmps.tile([NC8, D], F32, tag="c3_smps")
            for n0 in (0, 512):
                nn = min(512, D - n0)
                nc.tensor.matmul(sm[:, n0:n0 + nn], lhsT=cx.cwT[b2],
                                 rhs=pmean[:, n0:n0 + nn], start=True, stop=True)
            smc = pool.tile([NC8, D], F32, tag="c3_smc")
            nc.scalar.activation(out=smc, in_=sm, func=ACTF.Copy, scale=1.0 / N)
            nc.sync.dma_start(out=cx.selin[b2 * NC8:(b2 + 1) * NC8, 0:D],
                              in_=smc)


def _build_sel(nc, tc, cx):
    """sel softmax [16,4] -> cx.selrep [128, 64]."""
    with ExitStack() as stk:
        pool = stk.enter_context(tc.tile_pool(name="selp", bufs=1))
        ps = stk.enter_context(tc.tile_pool(name="selp_ps", bufs=2, space="PSUM"))
        selin = cx.selin
        selinT = pool.tile([P, K2D, 2 * NC8], F32R)
        for kb in range(K2D):
            pt = ps.tile([P, 2 * NC8], F32, tag="h_ptsel")
            nc.tensor.transpose(pt, selin[:, kb * P:(kb + 1) * P],
                                cx.ident[:2 * NC8, :2 * NC8])
            nc.any.tensor_copy(out=selinT[:, kb, :], in_=pt)
        swT = pool.tile([P, K2D, 4], F32R)
        nc.sync.dma_start(out=swT,
                          in_=cx.t_selwT.rearrange("(ks p) o -> p ks o", p=P))
        sp = ps.tile([2 * NC8, 4], F32, tag="h_selps")
        _gemm(nc, sp, lambda ks: selinT[:, ks, :], swT, K2D, 4, mrows=2 * NC8)
        sel = pool.tile([2 * NC8, 4], F32)
        sbbc = pool.tile([2 * NC8, 4], F32)
        nc.gpsimd.dma_start(out=sbbc, in_=_bcast_ap(cx.t_selb, 2 * NC8))
        nc.vector.tensor_add(out=sel, in0=sp, in1=sbbc)
        _softmax_rows(nc, pool, sel[:, :], 2 * NC8, "h_sel")
        nc.sync.dma_start(out=cx.t_selsc[:, :], in_=sel)
        selflat = pool.tile([1, 64], F32)
        nc.sync.dma_start(out=selflat,
                          in_=cx.t_selsc.rearrange("b f -> (b f)")[None, :])
        selrep = cx.glob.tile([P, 64], F32, tag="selrep")
        nc.gpsimd.partition_broadcast(selrep, selflat)
    cx.selrep = selrep


def _build_operators(nc, tc, cx):
    """4 operator MLPs over combT rows; sel-weighted accumulate to operated."""
    with ExitStack() as stk:
        wpool = stk.enter_context(tc.tile_pool(name="op_w", bufs=1))
        lpool = stk.enter_context(tc.tile_pool(name="op_l", bufs=3))
        work = stk.enter_context(tc.tile_pool(name="op_work", bufs=2))
        stat = stk.enter_context(tc.tile_pool(name="op_stat", bufs=4))
        ps1 = stk.enter_context(tc.tile_pool(name="op_ps1", bufs=4, space="PSUM"))
        ps2 = stk.enter_context(tc.tile_pool(name="op_ps2", bufs=1, space="PSUM"))
        pst = stk.enter_context(tc.tile_pool(name="op_pst", bufs=2, space="PSUM"))

        general = not cx.op_trivial
        nchunks = ROWS // P
        for k in range(4):
            w1 = wpool.tile([P, KD, 2 * D], F32R, tag="op_w1")
            nc.sync.dma_start(
                out=w1, in_=cx.t_op[k]["w1T"].rearrange("(ks p) o -> p ks o", p=P))
            w2 = wpool.tile([P, K2D, D], F32R, tag="op_w2")
            nc.sync.dma_start(
                out=w2, in_=cx.t_op[k]["w2T"].rearrange("(ks p) o -> p ks o", p=P))
            gb = {}
            if general:
                for nm, width in (("b1", 2 * D), ("ln1w", 2 * D),
                                  ("ln1b", 2 * D), ("b2", D), ("ln2w", D),
                                  ("ln2b", D)):
                    t = wpool.tile([P, width], F32, tag=f"op_g_{nm}")
                    nc.gpsimd.dma_start(out=t, in_=_bcast_ap(cx.t_op[k][nm], P))
                    gb[nm] = t

            for c in range(nchunks):
                lhs = lpool.tile([P, KD, P], F32R, tag="op_lhs")
                nc.sync.dma_start(out=lhs,
                                  in_=cx.t_combT[:, :, c * P:(c + 1) * P]
                                  .rearrange("ks p r -> p ks r"))
                y1 = []
                for n0 in range(0, 2 * D, 512):
                    py = ps1.tile([P, 512], F32, tag="op_y1")
                    for ks in range(KD):
                        nc.tensor.matmul(py, lhsT=lhs[:, ks, :],
                                         rhs=w1[:, ks, n0:n0 + 512],
                                         start=(ks == 0), stop=(ks == KD - 1))
                    y1.append(py)
                if general:
                    y1s = work.tile([P, 2 * D], F32, tag="op_y1g")
                    for si, py in enumerate(y1):
                        nc.vector.tensor_add(
                            out=y1s[:, si * 512:(si + 1) * 512], in0=py,
                            in1=gb["b1"][:, si * 512:(si + 1) * 512])
                    srcs = [y1s[:, i * 512:(i + 1) * 512] for i in range(3)]
                else:
                    srcs = y1
                mean, rstd = _ln_stats(nc, cx, stat, srcs)
                xh1 = work.tile([P, 2 * D], F32, tag="op_xh1")
                for si, sl in enumerate(srcs):
                    nc.vector.tensor_scalar(
                        out=xh1[:, si * 512:(si + 1) * 512], in0=sl,
                        scalar1=mean, scalar2=rstd,
                        op0=ALU.subtract, op1=ALU.mult)
                if general:
                    nc.vector.tensor_mul(out=xh1, in0=xh1, in1=gb["ln1w"])
                    nc.vector.tensor_add(out=xh1, in0=xh1, in1=gb["ln1b"])
                hT = work.tile([P, K2D, P], F32R, tag="op_hT")
                _tp128(nc, cx, pst, hT, xh1, K2D, relu=True)
                y2 = ps2.tile([P, D], F32, tag="op_y2")
                _gemm(nc, y2, lambda ks: hT[:, ks, :], w2, K2D, D)
                if general:
                    y2s = work.tile([P, D], F32, tag="op_y2g")
                    nc.vector.tensor_add(out=y2s, in0=y2, in1=gb["b2"])
                    s2 = [y2s[:, 0:512], y2s[:, 512:768]]
                else:
                    s2 = [y2[:, 0:512], y2[:, 512:768]]
                mean2, rstd2 = _ln_stats(nc, cx, stat, s2)
                selsl = cx.selrep[:, (c // 2) * 4 + k:(c // 2) * 4 + k + 1]
                contrib = work.tile([P, D], F32, tag="op_contrib")
                if general:
                    xh2 = work.tile([P, D], F32, tag="op_xh2")
                    nc.vector.tensor_scalar(out=xh2, in0=y2s, scalar1=mean2,
                                            scalar2=rstd2, op0=ALU.subtract,
                                            op1=ALU.mult)
                    nc.vector.tensor_mul(out=xh2, in0=xh2, in1=gb["ln2w"])
                    nc.vector.tensor_add(out=xh2, in0=xh2, in1=gb["ln2b"])
                    nc.vector.tensor_scalar_mul(contrib, xh2, selsl)
                else:
                    selsc = stat.tile([P, 1], F32, tag="op_selsc")
                    nc.vector.tensor_mul(out=selsc, in0=rstd2, in1=selsl)
                    for sl, w0, w1_ in ((s2[0], 0, 512), (s2[1], 512, 768)):
                        nc.vector.tensor_scalar(out=contrib[:, w0:w1_], in0=sl,
                                                scalar1=mean2, scalar2=selsc,
                                                op0=ALU.subtract, op1=ALU.mult)
                if k == 0:
                    nc.sync.dma_start(out=cx.t_oper[c * P:(c + 1) * P, :],
                                      in_=contrib)
                else:
                    nc.gpsimd.dma_start(out=cx.t_oper[c * P:(c + 1) * P, :],
                                        in_=contrib, accum_op=ALU.add)


def _build_composer(nc, tc, cx):
    """21 chained steps; writes out0/1/2 and per-batch gate sigmoids."""
    with ExitStack() as stk:
        fpool = stk.enter_context(tc.tile_pool(name="cp_f", bufs=1))
        wpool = stk.enter_context(tc.tile_pool(name="cp_w", bufs=1))
        work = stk.enter_context(tc.tile_pool(name="cp_work", bufs=2))
        stat = stk.enter_context(tc.tile_pool(name="cp_stat", bufs=4))
        ps = stk.enter_context(tc.tile_pool(name="cp_ps", bufs=2, space="PSUM"))
        pst = stk.enter_context(tc.tile_pool(name="cp_pst", bufs=2, space="PSUM"))
        psg = stk.enter_context(tc.tile_pool(name="cp_psg", bufs=1, space="PSUM"))

        general = not cx.comp_trivial

        with tc.tile_pool(name="cp_init", bufs=2) as ipool:
            for i in range(NC8):
                st = ipool.tile([P, KD, CROWS], F32R, tag="cp_st")
                for b2 in range(BL):
                    frm = ipool.tile([P, 2, D], F32, tag="cp_frm")
                    r0 = b2 * (NC8 * N) + i * N
                    nc.sync.dma_start(out=frm,
                                      in_=cx.t_oper[r0:r0 + N, :]
                                      .rearrange("(c p) d -> p c d", p=P))
                    for cc in range(2):
                        _tp128(nc, cx, pst, st, frm[:, cc, :], KD,
                               dst_col0=b2 * N + cc * P)
                nc.sync.dma_start(out=cx.t_fTd[i], in_=st)

        def load_fT(i):
            t = work.tile([P, KD, CROWS], F32R, tag="cp_fload")
            nc.sync.dma_start(out=t, in_=cx.t_fTd[i])
            return t

        cont_w = fpool.tile([1, D], F32)
        nc.sync.dma_start(out=cont_w, in_=cx.t_contw[:, :])
        cwbc = fpool.tile([P, D], F32)
        nc.gpsimd.partition_broadcast(cwbc, cont_w)
        contb = fpool.tile([1, 1], F32)
        nc.sync.dma_start(out=contb, in_=cx.t_contb[:, :])
        contbrep = fpool.tile([P, 1], F32)
        nc.gpsimd.partition_broadcast(contbrep, contb)

        newT = load_fT(0)
        for depth in range(3):
            wT = wpool.tile([P, K2D, D], F32R, tag="cp_wT")
            nc.sync.dma_start(
                out=wT, in_=cx.t_cp[depth]["wT"].rearrange("(ks p) o -> p ks o", p=P))
            gb = {}
            if general:
                for nm in ("b", "lnw", "lnb"):
                    t = wpool.tile([P, D], F32, tag=f"cp_g_{nm}")
                    nc.gpsimd.dma_start(out=t,
                                        in_=_bcast_ap(cx.t_cp[depth][nm], P))
                    gb[nm] = t
            new_rm = None
            for fi in range(1, NC8):
                out_is_last = (fi == NC8 - 1)
                fTt = load_fT(fi)
                nextT = work.tile([P, KD, CROWS], F32R, tag="cp_nextT")
                if out_is_last:
                    new_rm = work.tile([P, 4, D], F32, tag="cp_newrm")
                for cch in range(4):
                    py = ps.tile([P, D], F32, tag="cp_py")
                    for n0 in (0, 512):
                        nn = min(512, D - n0)
                        for ks in range(K2D):
                            lhsT = (newT[:, ks, cch * P:(cch + 1) * P]
                                    if ks < KD else
                                    fTt[:, ks - KD, cch * P:(cch + 1) * P])
                            nc.tensor.matmul(py[:, n0:n0 + nn], lhsT=lhsT,
                                             rhs=wT[:, ks, n0:n0 + nn],
                                             start=(ks == 0),
                                             stop=(ks == K2D - 1))
                    if general:
                        yg = work.tile([P, D], F32, tag="cp_yg")
                        nc.vector.tensor_add(out=yg, in0=py, in1=gb["b"])
                        s2 = [yg[:, 0:512], yg[:, 512:768]]
                    else:
                        s2 = [py[:, 0:512], py[:, 512:768]]
                    mean, rstd = _ln_stats(nc, cx, stat, s2)
                    xh = work.tile([P, D], F32, tag="cp_xh")
                    for sl, w0, w1_ in ((s2[0], 0, 512), (s2[1], 512, 768)):
                        nc.vector.tensor_scalar(out=xh[:, w0:w1_], in0=sl,
                                                scalar1=mean, scalar2=rstd,
                                                op0=ALU.subtract, op1=ALU.mult)
                    relu_done = False
                    if general:
                        nc.vector.tensor_mul(out=xh, in0=xh, in1=gb["lnw"])
                        nc.vector.tensor_add(out=xh, in0=xh, in1=gb["lnb"])
                        nc.vector.tensor_scalar_max(xh, xh, 0.0)
                        relu_done = True
                    for kb in range(KD):
                        pt = pst.tile([P, 4 * P], F32, tag="trp")
                        nc.tensor.transpose(pt[:, 0:P],
                                            xh[:, kb * P:(kb + 1) * P],
                                            cx.ident)
                        nc.scalar.activation(
                            out=nextT[:, kb, cch * P:(cch + 1) * P],
                            in_=pt[:, 0:P],
                            func=(ACTF.Copy if relu_done else ACTF.Relu))
                    if out_is_last:
                        if relu_done:
                            nc.any.tensor_copy(out=new_rm[:, cch, :], in_=xh)
                        else:
                            nc.vector.tensor_scalar_max(new_rm[:, cch, :],
                                                        xh, 0.0)
                newT = nextT
            outt = (cx.t_out0, cx.t_out1, cx.t_out2)[depth]
            nc.sync.dma_start(out=outt.rearrange("(c p) d -> p c d", p=P),
                              in_=new_rm)
            if depth < 2:
                gp = psg.tile([P, D], F32, tag="cp_gp")
                for b2 in range(BL):
                    for n0 in (0, 512):
                        nn = min(512, D - n0)
                        for cch in range(2):
                            nc.tensor.matmul(
                                gp[b2 * 64:b2 * 64 + 1, n0:n0 + nn],
                                lhsT=cx.ones_col,
                                rhs=new_rm[:, b2 * 2 + cch, n0:n0 + nn],
                                start=(cch == 0), stop=(cch == 1))
                gm = stat.tile([P, D], F32, tag="cp_gm")
                nc.scalar.activation(out=gm, in_=gp, func=ACTF.Copy,
                                     scale=1.0 / N)
                nc.vector.tensor_mul(out=gm, in0=gm, in1=cwbc)
                gv = stat.tile([P, 1], F32, tag="cp_gv")
                nc.vector.tensor_reduce(out=gv, in_=gm, axis=AX.X, op=ALU.add)
                gs = stat.tile([P, 1], F32, tag="cp_gs")
                nc.scalar.activation(out=gs, in_=gv, func=ACTF.Sigmoid,
                                     bias=contbrep, scale=1.0)
                for b2 in range(BL):
                    nc.sync.dma_start(
                        out=cx.t_gates[depth:depth + 1, b2:b2 + 1],
                        in_=gs[b2 * 64:b2 * 64 + 1, :])


def build(flags):
    nc = bacc.Bacc("TRN2", target_bir_lowering=False, debug=False,
                   enable_asserts=False)
    cx = Ctx()
    cx.enc_trivial, cx.op_trivial, cx.comp_trivial = flags

    def din(name, shape, dt=F32):
        return nc.dram_tensor(name, shape, dt, kind="ExternalInput").ap()

    cx.t_comp = din("comp", [ROWS, D])
    cx.t_text = din("text", [P, D])
    cx.t_maskb = din("maskb", [BL * 64])
    cx.t_enc = []
    for li in range(3):
        d = {}
        for nm, shape in (("qkvwT", [D, 3 * D]), ("outwT", [D, D]),
                          ("ff1wT", [D, FF]), ("ff2wT", [FF, D])):
            d[nm] = din(f"enc{li}_{nm}", shape, F32R)
        for nm, shape in (("qkvb", [3 * D]), ("outb", [D]), ("ff1b", [FF]),
                          ("ff2b", [D]), ("ln1w", [D]), ("ln1b", [D]),
                          ("ln2w", [D]), ("ln2b", [D])):
            d[nm] = din(f"enc{li}_{nm}", shape)
        cx.t_enc.append(d)
    cx.t_og_w1T = din("og_w1T", [D, 2 * D], F32R)
    cx.t_og_w2T = din("og_w2T", [2 * D, 64], F32R)
    cx.t_og_b1 = din("og_b1", [2 * D])
    cx.t_og_b2 = din("og_b2", [64])
    cx.t_selwT = din("selwT", [2 * D, 4], F32R)
    cx.t_selb = din("selb", [4])
    cx.t_op = []
    for k in range(4):
        d = {"w1T": din(f"op{k}_w1T", [D, 2 * D], F32R),
             "w2T": din(f"op{k}_w2T", [2 * D, D], F32R)}
        for nm, shape in (("b1", [2 * D]), ("ln1w", [2 * D]), ("ln1b", [2 * D]),
                          ("b2", [D]), ("ln2w", [D]), ("ln2b", [D])):
            d[nm] = din(f"op{k}_{nm}", shape)
        cx.t_op.append(d)
    cx.t_cp = []
    for dd in range(3):
        d = {"wT": din(f"cp{dd}_wT", [2 * D, D], F32R)}
        for nm in ("b", "lnw", "lnb"):
            d[nm] = din(f"cp{dd}_{nm}", [D])
        cx.t_cp.append(d)
    cx.t_contw = din("contw", [1, D])
    cx.t_contb = din("contb", [1, 1])

    cx.t_combT = nc.dram_tensor("combT", [KD, P, ROWS], F32R).ap()
    cx.t_oper = nc.dram_tensor("operated", [ROWS, D], F32).ap()
    cx.t_fTd = nc.dram_tensor("fTd", [NC8, P, KD, CROWS], F32R).ap()
    cx.t_cmsc = nc.dram_tensor("cmsc", [BL, 64], F32).ap()
    cx.t_selsc = nc.dram_tensor("selsc", [2 * NC8, 4], F32).ap()

    cx.t_out0 = nc.dram_tensor("out0", [CROWS, D], F32, kind="ExternalOutput").ap()
    cx.t_out1 = nc.dram_tensor("out1", [CROWS, D], F32, kind="ExternalOutput").ap()
    cx.t_out2 = nc.dram_tensor("out2", [CROWS, D], F32, kind="ExternalOutput").ap()
    cx.t_gates = nc.dram_tensor("gates", [2, BL], F32, kind="ExternalOutput").ap()

    with tile.TileContext(nc) as tc:
        with ExitStack() as stk:
            consts = stk.enter_context(tc.tile_pool(name="consts", bufs=1))
            cx.glob = stk.enter_context(tc.tile_pool(name="glob", bufs=1))
            cx.ident = consts.tile([P, P], F32)
            make_identity(nc, cx.ident)
            cx.eps = consts.tile([P, 1], F32)
            nc.vector.memset(cx.eps, 1e-5)
            cx.ones_col = consts.tile([P, 1], F32)
            nc.vector.memset(cx.ones_col, 1.0)

            _build_encoder(nc, tc, cx)
            _build_head(nc, tc, cx)
            _build_combined(nc, tc, cx)
            _build_sel(nc, tc, cx)
            _build_operators(nc, tc, cx)
            _build_composer(nc, tc, cx)
    nc.compile()
    return nc


# ------------------------------------------------------------------ host API
def _trivial(vecs_ones, vecs_zeros):
    return (all(np.all(np.asarray(v) == 1.0) for v in vecs_ones)
            and all(np.all(np.asarray(v) == 0.0) for v in vecs_zeros))


def kernel(components, text_feats, text_mask, params):
    components = np.asarray(components, dtype=np.float32)
    text_feats = np.asarray(text_feats, dtype=np.float32)
    text_mask = np.asarray(text_mask)
    pa = params

    enc_trivial = _trivial(
        [p[f"ln{i}_w"] for p in pa["enc"] for i in (1, 2)],
        [p[f"ln{i}_b"] for p in pa["enc"] for i in (1, 2)]
        + [p[nm] for p in pa["enc"] for nm in ("qkv_b", "out_b", "lin1_b",
                                               "lin2_b")])
    op_trivial = _trivial(
        [p[f"ln{i}_w"] for p in pa["operators"] for i in (1, 2)],
        [p[f"ln{i}_b"] for p in pa["operators"] for i in (1, 2)]
        + [p[nm] for p in pa["operators"] for nm in ("b1", "b2")])
    comp_trivial = _trivial(
        [lp["ln_w"] for lp in pa["composer"]["layers"]],
        [lp["ln_b"] for lp in pa["composer"]["layers"]]
        + [lp["b"] for lp in pa["composer"]["layers"]])

    flags = (enc_trivial, op_trivial, comp_trivial)
    if flags not in _CACHE:
        _CACHE[flags] = build(flags)
    nc = _CACHE[flags]

    f32 = lambda x: np.ascontiguousarray(np.asarray(x), dtype=np.float32)
    shared = {}
    for li, p in enumerate(pa["enc"]):
        shared[f"enc{li}_qkvwT"] = f32(np.asarray(p["qkv_w"]).T)
        shared[f"enc{li}_outwT"] = f32(np.asarray(p["out_w"]).T)
        shared[f"enc{li}_ff1wT"] = f32(np.asarray(p["lin1_w"]).T)
        shared[f"enc{li}_ff2wT"] = f32(np.asarray(p["lin2_w"]).T)
        shared[f"enc{li}_qkvb"] = f32(p["qkv_b"])
        shared[f"enc{li}_outb"] = f32(p["out_b"])
        shared[f"enc{li}_ff1b"] = f32(p["lin1_b"])
        shared[f"enc{li}_ff2b"] = f32(p["lin2_b"])
        shared[f"enc{li}_ln1w"] = f32(p["ln1_w"])
        shared[f"enc{li}_ln1b"] = f32(p["ln1_b"])
        shared[f"enc{li}_ln2w"] = f32(p["ln2_w"])
        shared[f"enc{li}_ln2b"] = f32(p["ln2_b"])
    g = pa["opgen"]
    shared["og_w1T"] = f32(np.asarray(g["w1"]).T)
    shared["og_w2T"] = f32(np.asarray(g["w2"]).T)
    shared["og_b1"] = f32(g["b1"])
    shared["og_b2"] = f32(g["b2"])
    shared["selwT"] = f32(np.asarray(pa["sel_w"]).T)
    shared["selb"] = f32(pa["sel_b"])
    for k, p in enumerate(pa["operators"]):
        shared[f"op{k}_w1T"] = f32(np.asarray(p["w1"]).T)
        shared[f"op{k}_w2T"] = f32(np.asarray(p["w2"]).T)
        shared[f"op{k}_b1"] = f32(p["b1"])
        shared[f"op{k}_b2"] = f32(p["b2"])
        shared[f"op{k}_ln1w"] = f32(p["ln1_w"])
        shared[f"op{k}_ln1b"] = f32(p["ln1_b"])
        shared[f"op{k}_ln2w"] = f32(p["ln2_w"])
        shared[f"op{k}_ln2b"] = f32(p["ln2_b"])
    rc = pa["composer"]
    for dd, lp in enumerate(rc["layers"]):
        shared[f"cp{dd}_wT"] = f32(np.asarray(lp["w"]).T)
        shared[f"cp{dd}_b"] = f32(lp["b"])
        shared[f"cp{dd}_lnw"] = f32(lp["ln_w"])
        shared[f"cp{dd}_lnb"] = f32(lp["ln_b"])
    shared["contw"] = f32(np.asarray(rc["cont_w"]).reshape(1, D))
    shared["contb"] = f32(np.asarray(rc["cont_b"]).reshape(1, 1))

    in_maps = []
    for c in range(NCORES):
        b0 = c * BL
        m = dict(shared)
        m["comp"] = f32(components[b0:b0 + BL].transpose(0, 2, 1, 3)
                        .reshape(ROWS, D))
        m["text"] = f32(text_feats[b0:b0 + BL].reshape(P, D))
        m["maskb"] = np.where(text_mask[b0:b0 + BL].reshape(-1),
                              np.float32(0.0), np.float32(-1e9))
        in_maps.append(m)

    res = run_bass_kernel_spmd(nc, in_maps, core_ids=list(range(NCORES)),
                               trace=TRACE)
    kernel.last_result = res

    gates = np.stack([res.results[c]["gates"] for c in range(NCORES)])
    p1 = np.float32(gates[:, 0, :].reshape(-1).astype(np.float32).sum() / B)
    p2 = np.float32(gates[:, 1, :].reshape(-1).astype(np.float32).sum() / B)
    active1 = bool(p1 >= 0.5)
    active2 = active1 and bool(p2 >= 0.5)
    name = "out2" if active2 else ("out1" if active1 else "out0")
    out = np.concatenate([res.results[c][name].reshape(BL, N, D)
                          for c in range(NCORES)], axis=0)
    return out
